# revision 4
# baseline (speedup 1.0000x reference)
"""GroupQLinear Trainium2 kernel.

y_q, y_delta, y_e = group_quant(dequant(x_q, x_delta, x_e) @ W.T + bias)

Strategy (8 NeuronCores, data-parallel over B*T=8192 tokens, 1024/core):
  - dequant: z[t,h] = x_q * 2^-x_e  (exact in fp16; per-token delta factors
    out of the matmul and is applied to the PSUM result instead)
  - matmul: 2-pass fp16 split weights (W = W_hi + W_lo, 11+11 mantissa bits
    ~ fp32 quality) at 1 cycle/row on the PE (~70 TFLOP/s/core measured)
  - re-quant: absmax/group-absmax reduces + exponent bit-tricks, all fp32,
    bit-matched to the jax reference semantics
"""
import sys
import numpy as np
import ml_dtypes

for p in ("/opt/trn_rl_repo", "/root/.axon_site/_ro/trn_rl_repo"):
    if p not in sys.path:
        sys.path.append(p)

import concourse.bacc as bacc
import concourse.tile as tile
from concourse import mybir
from concourse.alu_op_type import AluOpType
from concourse.bass_utils import run_bass_kernel_spmd

dt = mybir.dt
AX = mybir.AxisListType

B, T, H, O = 4, 2048, 4096, 4096
NCORES = 8
TOK = B * T                 # 8192 tokens
TPC = TOK // NCORES         # 1024 tokens per core
KC = H // 128               # 32 k-chunks
TT = TPC // 128             # 8 token tiles per core
TGS = 2                     # token tiles per group (PSUM: 2*2 banks, x2 bufs)
NTG = TT // TGS             # 4 groups
OQ = 4                      # o-quarters
OQW = O // OQ               # 1024
GRP = 32                    # quant group width
GO = O // GRP               # 128 output groups

_CACHE = {}


def _build():
    nc = bacc.Bacc("TRN2", target_bir_lowering=False, debug=False,
                   num_devices=NCORES)
    xq = nc.dram_tensor("xq", [H, TPC], dt.int8, kind="ExternalInput").ap()
    xe = nc.dram_tensor("xe", [128, TPC], dt.int8, kind="ExternalInput").ap()
    dl = nc.dram_tensor("dl", [TPC], dt.float32, kind="ExternalInput").ap()
    whi = nc.dram_tensor("whi", [H, O], dt.float16, kind="ExternalInput").ap()
    wlo = nc.dram_tensor("wlo", [H, O], dt.float16, kind="ExternalInput").ap()
    bi = nc.dram_tensor("bi", [O], dt.float32, kind="ExternalInput").ap()
    yq = nc.dram_tensor("yq", [TPC, O], dt.int8, kind="ExternalOutput").ap()
    yd = nc.dram_tensor("yd", [TPC], dt.float32, kind="ExternalOutput").ap()
    ye = nc.dram_tensor("ye", [TPC, GO], dt.int8, kind="ExternalOutput").ap()
    p2s = nc.dram_tensor("p2s", [128, TPC], dt.float16).ap()  # scratch

    with tile.TileContext(nc) as tc:
        with tc.tile_pool(name="cst", bufs=1) as cst, \
             tc.tile_pool(name="zp", bufs=1) as zp, \
             tc.tile_pool(name="yp", bufs=1) as yp, \
             tc.tile_pool(name="wp", bufs=3) as wp, \
             tc.tile_pool(name="qp", bufs=2) as qp, \
             tc.tile_pool(name="spA", bufs=1) as spA, \
             tc.tile_pool(name="spB", bufs=2) as spB, \
             tc.tile_pool(name="ps", bufs=2, space="PSUM") as ps:

            # ---- constants ----
            bias_rep = cst.tile([128, O], dt.float32, tag="bias")
            nc.sync.dma_start(
                bias_rep[:],
                bi[:].rearrange("(a o) -> a o", a=1).to_broadcast((128, O)))

            # ---- 2^-e for the input exponents (small, then DRAM-expanded) ----
            xet = spA.tile([128, TPC], dt.int8, tag="xet")
            nc.sync.dma_start(xet[:], xe[:])
            e32 = spA.tile([128, TPC], dt.int32, tag="e32")
            nc.vector.tensor_scalar(e32[:], xet[:], -1, 127,
                                    AluOpType.mult, AluOpType.add)
            nc.vector.tensor_scalar(e32[:], e32[:], 23, None,
                                    AluOpType.logical_shift_left)
            p2t = spA.tile([128, TPC], dt.float16, tag="p2t")
            nc.vector.tensor_copy(p2t[:], e32[:].bitcast(dt.float32))
            nc.sync.dma_start(p2s[:], p2t[:])

            # ---- dequant: zT[k] = q * 2^-e  (fp16, exact) ----
            zts = [zp.tile([128, TPC], dt.float16, tag=f"z{k}", name=f"z{k}")
                   for k in range(KC)]
            for k in range(KC):
                qch = qp.tile([128, TPC], dt.int8, tag="qch")
                nc.sync.dma_start(qch[:], xq[k * 128:(k + 1) * 128, :])
                qf = qp.tile([128, TPC], dt.float16, tag="qf")
                nc.vector.tensor_copy(qf[:], qch[:])
                p2x = qp.tile([128, TPC], dt.float16, tag="p2x")
                src = p2s[4 * k:4 * k + 4, :] \
                    .rearrange("g (a t) -> g a t", a=1) \
                    .to_broadcast((4, GRP, TPC))
                nc.sync.dma_start(p2x[:], src)
                nc.vector.tensor_mul(zts[k][:], qf[:], p2x[:])

            # ---- per-token-tile delta tiles ----
            dts = []
            for tt in range(TT):
                dtile = cst.tile([128, 1], dt.float32, tag=f"dl{tt}")
                nc.sync.dma_start(dtile[:], dl[tt * 128:(tt + 1) * 128]
                                  .rearrange("(p a) -> p a", a=1))
                dts.append(dtile)

            # ---- main loop: matmul (2-pass fp16) + evacuate + quantize ----
            for tg in range(NTG):
                ysb = [yp.tile([128, O], dt.float32, tag=f"y{tt}", name=f"y{tt}")
                       for tt in range(TGS)]
                for oq in range(OQ):
                    ptiles = [ps.tile([128, 512], dt.float32, tag=f"p{i}", name=f"p{i}")
                              for i in range(2 * TGS)]
                    for k in range(KC):
                        wh = wp.tile([128, OQW], dt.float16, tag="wh")
                        nc.sync.dma_start(
                            wh[:], whi[k * 128:(k + 1) * 128,
                                       oq * OQW:(oq + 1) * OQW])
                        wl = wp.tile([128, OQW], dt.float16, tag="wl")
                        nc.sync.dma_start(
                            wl[:], wlo[k * 128:(k + 1) * 128,
                                       oq * OQW:(oq + 1) * OQW])
                        for tt in range(TGS):
                            lhs = zts[k][:, (tg * TGS + tt) * 128:
                                         (tg * TGS + tt + 1) * 128]
                            for nb in range(2):
                                pt = ptiles[tt * 2 + nb]
                                rhs_h = wh[:, nb * 512:(nb + 1) * 512]
                                rhs_l = wl[:, nb * 512:(nb + 1) * 512]
                                nc.tensor.matmul(pt[:], lhs, rhs_h,
                                                 start=(k == 0), stop=False)
                                nc.tensor.matmul(pt[:], lhs, rhs_l,
                                                 start=False,
                                                 stop=(k == KC - 1))
                    # evacuate PSUM: y = psum * delta + bias
                    for tt in range(TGS):
                        for nb in range(2):
                            osl = np.s_[:, oq * OQW + nb * 512:
                                        oq * OQW + (nb + 1) * 512]
                            nc.vector.scalar_tensor_tensor(
                                ysb[tt][osl], ptiles[tt * 2 + nb][:],
                                dts[tg * TGS + tt][:], bias_rep[osl],
                                AluOpType.mult, AluOpType.add)

                # ---- quantize each finished token tile ----
                for tt in range(TGS):
                    t0 = (tg * TGS + tt) * 128
                    y = ysb[tt]
                    am = spB.tile([128, 1], dt.float32, tag="am")
                    nc.vector.tensor_reduce(am[:], y[:], axis=AX.X,
                                            op=AluOpType.max,
                                            apply_absolute_value=True)
                    gm = spB.tile([128, GO], dt.float32, tag="gm")
                    yv = y[:].rearrange("p (g w) -> p g w", w=GRP)
                    nc.vector.tensor_reduce(gm[:], yv, axis=AX.X,
                                            op=AluOpType.max,
                                            apply_absolute_value=True)
                    # delta_y = amax/127 (1/127 const; recip correctly rounded)
                    dy = spB.tile([128, 1], dt.float32, tag="dy")
                    nc.vector.tensor_scalar(dy[:], am[:],
                                            float(np.float32(1.0 / 127.0)), None,
                                            AluOpType.mult)
                    nc.sync.dma_start(
                        yd[t0:t0 + 128].rearrange("(p a) -> p a", a=1), dy[:])
                    # numerator n = delta_y * 127 (match reference rounding)
                    nn = spB.tile([128, 1], dt.float32, tag="nn")
                    nc.vector.tensor_scalar(nn[:], dy[:], 127.0,
                                            None, AluOpType.mult)
                    # ratio = n / max(gmax, eps), clipped to >= 1
                    gmc = spB.tile([128, GO], dt.float32, tag="gmc")
                    nc.vector.tensor_scalar(gmc[:], gm[:],
                                            1e-8, None,
                                            AluOpType.max)
                    rg = spB.tile([128, GO], dt.float32, tag="rg")
                    nc.vector.reciprocal(rg[:], gmc[:])
                    rt = spB.tile([128, GO], dt.float32, tag="rt")
                    nc.vector.tensor_scalar(rt[:], rg[:], nn[:, 0:1],
                                            1.0,
                                            AluOpType.mult, AluOpType.max)
                    # e = min((bits >> 23) - 127, 15)
                    ei = spB.tile([128, GO], dt.int32, tag="ei")
                    nc.vector.tensor_scalar(ei[:], rt[:].bitcast(dt.int32),
                                            23, None,
                                            AluOpType.logical_shift_right)
                    ec = spB.tile([128, GO], dt.int32, tag="ec")
                    nc.vector.tensor_scalar(ec[:], ei[:], 127, 15,
                                            AluOpType.subtract, AluOpType.min)
                    e8 = spB.tile([128, GO], dt.int8, tag="e8")
                    nc.vector.tensor_copy(e8[:], ec[:])
                    nc.sync.dma_start(ye[t0:t0 + 128, :], e8[:])
                    # rm = 1 / max(delta_y * 2^-e, eps)
                    p2i = spB.tile([128, GO], dt.int32, tag="p2i")
                    nc.vector.tensor_scalar(p2i[:], ec[:], -1, 127,
                                            AluOpType.mult, AluOpType.add)
                    nc.vector.tensor_scalar(p2i[:], p2i[:], 23, None,
                                            AluOpType.logical_shift_left)
                    scl = spB.tile([128, GO], dt.float32, tag="scl")
                    nc.vector.tensor_scalar(scl[:], p2i[:].bitcast(dt.float32),
                                            dy[:, 0:1], 1e-8,
                                            AluOpType.mult, AluOpType.max)
                    rm = spB.tile([128, GO], dt.float32, tag="rm")
                    nc.vector.reciprocal(rm[:], scl[:])
                    # q = convert_rne(y * rm)  (saturating int8; |q| <= 127.1)
                    qf32 = spA.tile([128, O], dt.float32, tag="qf32")
                    rmb = rm[:].rearrange("p (g a) -> p g a", a=1) \
                        .to_broadcast((128, GO, GRP))
                    nc.vector.tensor_mul(
                        qf32[:].rearrange("p (g w) -> p g w", w=GRP), yv, rmb)
                    q8 = spB.tile([128, O], dt.int8, tag="q8")
                    nc.vector.tensor_copy(q8[:], qf32[:])
                    nc.sync.dma_start(yq[t0:t0 + 128, :], q8[:])

    nc.compile()
    return nc


def _prep(x_q, x_delta, x_e, weight, bias):
    xqT = np.ascontiguousarray(
        x_q.reshape(TOK, H).astype(np.int8).T)              # [H, TOK]
    xeT = np.ascontiguousarray(
        x_e.reshape(TOK, H // GRP).astype(np.int8).T)       # [128, TOK]
    dl = np.ascontiguousarray(np.asarray(x_delta).reshape(TOK)
                              .astype(np.float32))
    wT = np.ascontiguousarray(np.asarray(weight).astype(np.float32).T)
    whi = wT.astype(np.float16)
    wlo = (wT - whi.astype(np.float32)).astype(np.float16)
    bi = np.ascontiguousarray(np.asarray(bias).reshape(O).astype(np.float32))
    maps = []
    for c in range(NCORES):
        sl = np.s_[c * TPC:(c + 1) * TPC]
        maps.append({
            "xq": np.ascontiguousarray(xqT[:, sl]),
            "xe": np.ascontiguousarray(xeT[:, sl]),
            "dl": dl[sl],
            "whi": whi,
            "wlo": wlo,
            "bi": bi,
        })
    return maps


def kernel(x_q, x_delta, x_e, weight, bias, _trace=False):
    if "nc" not in _CACHE:
        _CACHE["nc"] = _build()
    nc = _CACHE["nc"]
    maps = _prep(np.asarray(x_q), np.asarray(x_delta), np.asarray(x_e),
                 np.asarray(weight), np.asarray(bias))
    res = run_bass_kernel_spmd(nc, maps, core_ids=list(range(NCORES)),
                               trace=_trace)
    if _trace:
        _CACHE["last_result"] = res
    y_q = np.empty((TOK, O), dtype=np.int8)
    y_d = np.empty((TOK,), dtype=np.float32)
    y_e = np.empty((TOK, GO), dtype=np.int8)
    for c in range(NCORES):
        sl = np.s_[c * TPC:(c + 1) * TPC]
        y_q[sl] = res.results[c]["yq"]
        y_d[sl] = res.results[c]["yd"].reshape(TPC)
        y_e[sl] = res.results[c]["ye"]
    return (y_q.reshape(B, T, O), y_d.reshape(B, T),
            y_e.reshape(B, T, GO))


# revision 5
# speedup vs baseline: 1.3161x; 1.3161x over previous
"""GroupQLinear Trainium2 kernel.

y_q, y_delta, y_e = group_quant(dequant(x_q, x_delta, x_e) @ W.T + bias)

Strategy (8 NeuronCores, data-parallel over B*T=8192 tokens, 1024/core):
  - dequant: z[t,h] = x_q * 2^-x_e  (exact in fp16; per-token delta factors
    out of the matmul and is applied to the PSUM result instead)
  - matmul: 2-pass fp16 split weights (W = W_hi + W_lo, 11+11 mantissa bits
    ~ fp32 quality) at 1 cycle/row on the PE (~70 TFLOP/s/core measured)
  - re-quant: absmax/group-absmax reduces + exponent bit-tricks, all fp32,
    bit-matched to the jax reference semantics
"""
import sys
import numpy as np
import ml_dtypes

for p in ("/opt/trn_rl_repo", "/root/.axon_site/_ro/trn_rl_repo"):
    if p not in sys.path:
        sys.path.append(p)

import concourse.bacc as bacc
import concourse.tile as tile
from concourse import mybir
from concourse.alu_op_type import AluOpType
from concourse.bass_utils import run_bass_kernel_spmd

dt = mybir.dt
AX = mybir.AxisListType

B, T, H, O = 4, 2048, 4096, 4096
NCORES = 8
TOK = B * T                 # 8192 tokens
TPC = TOK // NCORES         # 1024 tokens per core
KC = H // 128               # 32 k-chunks
TT = TPC // 128             # 8 token tiles per core
TGS = 2                     # token tiles per group (PSUM: 2*2 banks, x2 bufs)
NTG = TT // TGS             # 4 groups
OQ = 4                      # o-quarters
OQW = O // OQ               # 1024
GRP = 32                    # quant group width
GO = O // GRP               # 128 output groups

_CACHE = {}


def _build():
    nc = bacc.Bacc("TRN2", target_bir_lowering=False, debug=False,
                   num_devices=NCORES)
    xq = nc.dram_tensor("xq", [H, TPC], dt.int8, kind="ExternalInput").ap()
    xe = nc.dram_tensor("xe", [128, TPC], dt.int8, kind="ExternalInput").ap()
    dl = nc.dram_tensor("dl", [TPC], dt.float32, kind="ExternalInput").ap()
    whi = nc.dram_tensor("whi", [H, O], dt.float16, kind="ExternalInput").ap()
    wlo = nc.dram_tensor("wlo", [H, O], dt.float16, kind="ExternalInput").ap()
    bi = nc.dram_tensor("bi", [O], dt.float32, kind="ExternalInput").ap()
    yq = nc.dram_tensor("yq", [TPC, O], dt.int8, kind="ExternalOutput").ap()
    yd = nc.dram_tensor("yd", [TPC], dt.float32, kind="ExternalOutput").ap()
    ye = nc.dram_tensor("ye", [TPC, GO], dt.int8, kind="ExternalOutput").ap()
    p2s = nc.dram_tensor("p2s", [128, TPC], dt.float16).ap()  # scratch

    with tile.TileContext(nc) as tc:
        with tc.tile_pool(name="cst", bufs=1) as cst, \
             tc.tile_pool(name="zp", bufs=1) as zp, \
             tc.tile_pool(name="yp", bufs=1) as yp, \
             tc.tile_pool(name="wp", bufs=6) as wp, \
             tc.tile_pool(name="qp", bufs=2) as qp, \
             tc.tile_pool(name="spA", bufs=1) as spA, \
             tc.tile_pool(name="spB", bufs=2) as spB, \
             tc.tile_pool(name="ps", bufs=2, space="PSUM") as ps:

            # ---- constants ----
            bias_rep = cst.tile([128, O], dt.float32, tag="bias")
            nc.sync.dma_start(
                bias_rep[:],
                bi[:].rearrange("(a o) -> a o", a=1).to_broadcast((128, O)))

            # ---- 2^-e for the input exponents (small, then DRAM-expanded) ----
            xet = spA.tile([128, TPC], dt.int8, tag="xet")
            nc.sync.dma_start(xet[:], xe[:])
            e32 = spA.tile([128, TPC], dt.int32, tag="e32")
            nc.vector.tensor_scalar(e32[:], xet[:], -1, 127,
                                    AluOpType.mult, AluOpType.add)
            nc.vector.tensor_scalar(e32[:], e32[:], 23, None,
                                    AluOpType.logical_shift_left)
            p2t = spA.tile([128, TPC], dt.float16, tag="p2t")
            nc.vector.tensor_copy(p2t[:], e32[:].bitcast(dt.float32))
            nc.sync.dma_start(p2s[:], p2t[:])

            # ---- dequant: zT[k] = q * 2^-e  (fp16, exact) ----
            zts = [zp.tile([128, TPC], dt.float16, tag=f"z{k}", name=f"z{k}")
                   for k in range(KC)]
            for k in range(KC):
                qch = qp.tile([128, TPC], dt.int8, tag="qch")
                nc.gpsimd.dma_start(qch[:], xq[k * 128:(k + 1) * 128, :])
                qf = qp.tile([128, TPC], dt.float16, tag="qf")
                nc.vector.tensor_copy(qf[:], qch[:])
                p2x = qp.tile([128, TPC], dt.float16, tag="p2x")
                src = p2s[4 * k:4 * k + 4, :] \
                    .rearrange("g (a t) -> g a t", a=1) \
                    .to_broadcast((4, GRP, TPC))
                nc.gpsimd.dma_start(p2x[:], src)
                nc.vector.tensor_mul(zts[k][:], qf[:], p2x[:])

            # ---- per-token-tile delta tiles ----
            dts = []
            for tt in range(TT):
                dtile = cst.tile([128, 1], dt.float32, tag=f"dl{tt}")
                nc.sync.dma_start(dtile[:], dl[tt * 128:(tt + 1) * 128]
                                  .rearrange("(p a) -> p a", a=1))
                dts.append(dtile)

            # ---- main loop: matmul (2-pass fp16) + evacuate + quantize ----
            for tg in range(NTG):
                ysb = [yp.tile([128, O], dt.float32, tag=f"y{tt}", name=f"y{tt}")
                       for tt in range(TGS)]
                for oq in range(OQ):
                    ptiles = [ps.tile([128, 512], dt.float32, tag=f"p{i}", name=f"p{i}")
                              for i in range(2 * TGS)]
                    for k in range(KC):
                        wh = wp.tile([128, OQW], dt.float16, tag="wh")
                        nc.sync.dma_start(
                            wh[:], whi[k * 128:(k + 1) * 128,
                                       oq * OQW:(oq + 1) * OQW])
                        wl = wp.tile([128, OQW], dt.float16, tag="wl")
                        nc.sync.dma_start(
                            wl[:], wlo[k * 128:(k + 1) * 128,
                                       oq * OQW:(oq + 1) * OQW])
                        for tt in range(TGS):
                            lhs = zts[k][:, (tg * TGS + tt) * 128:
                                         (tg * TGS + tt + 1) * 128]
                            for nb in range(2):
                                pt = ptiles[tt * 2 + nb]
                                rhs_h = wh[:, nb * 512:(nb + 1) * 512]
                                rhs_l = wl[:, nb * 512:(nb + 1) * 512]
                                nc.tensor.matmul(pt[:], lhs, rhs_h,
                                                 start=(k == 0), stop=False)
                                nc.tensor.matmul(pt[:], lhs, rhs_l,
                                                 start=False,
                                                 stop=(k == KC - 1))
                    # evacuate PSUM: y = psum * delta + bias
                    for tt in range(TGS):
                        for nb in range(2):
                            osl = np.s_[:, oq * OQW + nb * 512:
                                        oq * OQW + (nb + 1) * 512]
                            nc.vector.scalar_tensor_tensor(
                                ysb[tt][osl], ptiles[tt * 2 + nb][:],
                                dts[tg * TGS + tt][:], bias_rep[osl],
                                AluOpType.mult, AluOpType.add)

                # ---- quantize each finished token tile ----
                for tt in range(TGS):
                    t0 = (tg * TGS + tt) * 128
                    y = ysb[tt]
                    am = spB.tile([128, 1], dt.float32, tag="am")
                    nc.vector.tensor_reduce(am[:], y[:], axis=AX.X,
                                            op=AluOpType.max,
                                            apply_absolute_value=True)
                    gm = spB.tile([128, GO], dt.float32, tag="gm")
                    yv = y[:].rearrange("p (g w) -> p g w", w=GRP)
                    nc.vector.tensor_reduce(gm[:], yv, axis=AX.X,
                                            op=AluOpType.max,
                                            apply_absolute_value=True)
                    # delta_y = amax/127 (1/127 const; recip correctly rounded)
                    dy = spB.tile([128, 1], dt.float32, tag="dy")
                    nc.vector.tensor_scalar(dy[:], am[:],
                                            float(np.float32(1.0 / 127.0)), None,
                                            AluOpType.mult)
                    nc.gpsimd.dma_start(
                        yd[t0:t0 + 128].rearrange("(p a) -> p a", a=1), dy[:])
                    # numerator n = delta_y * 127 (match reference rounding)
                    nn = spB.tile([128, 1], dt.float32, tag="nn")
                    nc.vector.tensor_scalar(nn[:], dy[:], 127.0,
                                            None, AluOpType.mult)
                    # ratio = n / max(gmax, eps), clipped to >= 1
                    gmc = spB.tile([128, GO], dt.float32, tag="gmc")
                    nc.vector.tensor_scalar(gmc[:], gm[:],
                                            1e-8, None,
                                            AluOpType.max)
                    rg = spB.tile([128, GO], dt.float32, tag="rg")
                    nc.vector.reciprocal(rg[:], gmc[:])
                    rt = spB.tile([128, GO], dt.float32, tag="rt")
                    nc.vector.tensor_scalar(rt[:], rg[:], nn[:, 0:1],
                                            1.0,
                                            AluOpType.mult, AluOpType.max)
                    # e = min((bits >> 23) - 127, 15)
                    ei = spB.tile([128, GO], dt.int32, tag="ei")
                    nc.vector.tensor_scalar(ei[:], rt[:].bitcast(dt.int32),
                                            23, None,
                                            AluOpType.logical_shift_right)
                    ec = spB.tile([128, GO], dt.int32, tag="ec")
                    nc.vector.tensor_scalar(ec[:], ei[:], 127, 15,
                                            AluOpType.subtract, AluOpType.min)
                    e8 = spB.tile([128, GO], dt.int8, tag="e8")
                    nc.vector.tensor_copy(e8[:], ec[:])
                    nc.gpsimd.dma_start(ye[t0:t0 + 128, :], e8[:])
                    # rm = 1 / max(delta_y * 2^-e, eps)
                    p2i = spB.tile([128, GO], dt.int32, tag="p2i")
                    nc.vector.tensor_scalar(p2i[:], ec[:], -1, 127,
                                            AluOpType.mult, AluOpType.add)
                    nc.vector.tensor_scalar(p2i[:], p2i[:], 23, None,
                                            AluOpType.logical_shift_left)
                    scl = spB.tile([128, GO], dt.float32, tag="scl")
                    nc.vector.tensor_scalar(scl[:], p2i[:].bitcast(dt.float32),
                                            dy[:, 0:1], 1e-8,
                                            AluOpType.mult, AluOpType.max)
                    rm = spB.tile([128, GO], dt.float32, tag="rm")
                    nc.vector.reciprocal(rm[:], scl[:])
                    # q = convert_rne(y * rm)  (saturating int8; |q| <= 127.1)
                    qf32 = spA.tile([128, O], dt.float32, tag="qf32")
                    rmb = rm[:].rearrange("p (g a) -> p g a", a=1) \
                        .to_broadcast((128, GO, GRP))
                    nc.vector.tensor_mul(
                        qf32[:].rearrange("p (g w) -> p g w", w=GRP), yv, rmb)
                    q8 = spB.tile([128, O], dt.int8, tag="q8")
                    nc.vector.tensor_copy(q8[:], qf32[:])
                    nc.gpsimd.dma_start(yq[t0:t0 + 128, :], q8[:])

    nc.compile()
    return nc


def _prep(x_q, x_delta, x_e, weight, bias):
    xqT = np.ascontiguousarray(
        x_q.reshape(TOK, H).astype(np.int8).T)              # [H, TOK]
    xeT = np.ascontiguousarray(
        x_e.reshape(TOK, H // GRP).astype(np.int8).T)       # [128, TOK]
    dl = np.ascontiguousarray(np.asarray(x_delta).reshape(TOK)
                              .astype(np.float32))
    wT = np.ascontiguousarray(np.asarray(weight).astype(np.float32).T)
    whi = wT.astype(np.float16)
    wlo = (wT - whi.astype(np.float32)).astype(np.float16)
    bi = np.ascontiguousarray(np.asarray(bias).reshape(O).astype(np.float32))
    maps = []
    for c in range(NCORES):
        sl = np.s_[c * TPC:(c + 1) * TPC]
        maps.append({
            "xq": np.ascontiguousarray(xqT[:, sl]),
            "xe": np.ascontiguousarray(xeT[:, sl]),
            "dl": dl[sl],
            "whi": whi,
            "wlo": wlo,
            "bi": bi,
        })
    return maps


def kernel(x_q, x_delta, x_e, weight, bias, _trace=False):
    if "nc" not in _CACHE:
        _CACHE["nc"] = _build()
    nc = _CACHE["nc"]
    maps = _prep(np.asarray(x_q), np.asarray(x_delta), np.asarray(x_e),
                 np.asarray(weight), np.asarray(bias))
    res = run_bass_kernel_spmd(nc, maps, core_ids=list(range(NCORES)),
                               trace=_trace)
    if _trace:
        _CACHE["last_result"] = res
    y_q = np.empty((TOK, O), dtype=np.int8)
    y_d = np.empty((TOK,), dtype=np.float32)
    y_e = np.empty((TOK, GO), dtype=np.int8)
    for c in range(NCORES):
        sl = np.s_[c * TPC:(c + 1) * TPC]
        y_q[sl] = res.results[c]["yq"]
        y_d[sl] = res.results[c]["yd"].reshape(TPC)
        y_e[sl] = res.results[c]["ye"]
    return (y_q.reshape(B, T, O), y_d.reshape(B, T),
            y_e.reshape(B, T, GO))
